# revision 6
# baseline (speedup 1.0000x reference)
"""LSTM decoder kernel for Trainium2 (8 NeuronCores, data-parallel over batch).

Reference computation (per batch element b):
    h0 = context_seq[b, -1, :]          # only the LAST timestep is used
    c0 = 0
    for t in range(T):
        gates = h @ (W_ih + W_hh).T + (b_ih + b_hh)     # [4H], order i,f,g,o
        i, f, g, o = split(gates)
        c = sigmoid(f) * c + sigmoid(i) * tanh(g)
        h = sigmoid(o) * tanh(c)
        pred[t] = h @ W_out.T + b_out                   # [O]

Device layout (per core, B=128 batch rows):
    state kept TRANSPOSED: hT, cT are [H=128 partitions, B=128 free].
    gates^T computed with 4 matmuls (stationary = weight block) into one
    PSUM tile [128, 4*B]; per-gate sigmoid/tanh applied with per-partition
    bias on ScalarE; elementwise updates on VectorE; per-step prediction
    via a small matmul (stationary = hT) giving pred [B, 7] naturally,
    accumulated into an SBUF buffer [128, T*7]; one DMA out at the end.
b_out is added on the host.
"""

import json

import numpy as np

B_TOTAL = 1024
H = 128
O = 7
N_CORES = 8
B_CORE = B_TOTAL // N_CORES  # 128


def _split_multiwait(bir_bytes: bytes) -> bytes:
    """This walrus build encodes at most ONE sync-wait per instruction.
    Split any multi-wait instruction into single-wait NoOps on the same
    engine (the sequencer executes them in program order, so waiting on
    each semaphore in turn is equivalent to waiting on all of them)."""
    bir = json.loads(bir_bytes)
    n = 0
    for f in bir.get("functions", []):
        for blk in f.get("blocks", []):
            new = []
            for inst in blk.get("instructions", []):
                si = inst.get("sync_info")
                waits = (si or {}).get("on_wait") or []
                if len(waits) > 1:
                    for w in waits[:-1]:
                        n += 1
                        nop = {
                            "name": f"WSPLIT-{n}",
                            "engine": inst.get("engine"),
                            "ins": [],
                            "outs": [],
                            "opcode": "NoOp",
                            "sync_info": {"on_update": [], "on_wait": [w]},
                        }
                        if inst.get("debug") is not None:
                            nop["debug"] = inst["debug"]
                        new.append(nop)
                    si["on_wait"] = [waits[-1]]
                new.append(inst)
            blk["instructions"] = new
    return json.dumps(bir).encode()


_PATCHED = False


def _patch_bass():
    global _PATCHED
    if _PATCHED:
        return
    import concourse.bass as bass

    orig = bass.Bass.to_json_bytes

    def patched(self, *a, **k):
        return _split_multiwait(orig(self, *a, **k))

    bass.Bass.to_json_bytes = patched
    _PATCHED = True


_PROGRAM_CACHE = {}

VARIANT = 1


def _build_program(T: int, variant: int = None):
    """Build the Bass/Tile program for T recurrence steps. Returns nc.

    variant 1: per-gate ACT with per-partition bias (5 ACT instrs/step).
    variant 2: gate order [i,f,o,g]; biases pre-added into PSUM with one
       K=4 one-hot matmul; merged sigmoid over i,f,o (3 ACT instrs/step).
       Expects host to supply wt in [i,f,o,g] order and a "bstack" input.
    """
    if variant is None:
        variant = VARIANT
    import concourse.bass as bass
    import concourse.tile as tile
    from concourse import mybir

    _patch_bass()

    fp32 = mybir.dt.float32
    AF = mybir.ActivationFunctionType

    nc = bass.Bass("TRN2", debug=False)
    # DRAM I/O (per-core shard shapes)
    d_h0t = nc.dram_tensor("h0t", [H, B_CORE], fp32, kind="ExternalInput").ap()
    d_wt = nc.dram_tensor("wt", [H, 4 * H], fp32, kind="ExternalInput").ap()
    d_bias = nc.dram_tensor("bias", [H, 4], fp32, kind="ExternalInput").ap()
    d_woutt = nc.dram_tensor("woutt", [H, O], fp32, kind="ExternalInput").ap()
    d_preds = nc.dram_tensor("preds", [B_CORE, T * O], fp32, kind="ExternalOutput").ap()

    with tile.TileContext(nc) as tc:
        with (
            tc.tile_pool(name="fixed", bufs=1) as fixed,
            tc.tile_pool(name="state", bufs=2) as state,
            tc.tile_pool(name="acts", bufs=2) as acts,
            tc.tile_pool(name="psum", bufs=2, space="PSUM") as psum_pool,
            tc.tile_pool(name="ppsum", bufs=2, space="PSUM") as ppsum_pool,
            tc.tile_pool(name="outp", bufs=1) as outp,
        ):
            wt = fixed.tile([H, 4 * H], fp32)
            nc.sync.dma_start(wt[:], d_wt[:])
            bias = fixed.tile([H, 4], fp32)
            nc.sync.dma_start(bias[:], d_bias[:])
            woutt = fixed.tile([H, O], fp32)
            nc.sync.dma_start(woutt[:], d_woutt[:])

            Bc = B_CORE
            if variant == 2:
                # bstack[k, m] = bias of gate k, row m (gate order i,f,o,g)
                # onehot[k, g*Bc + b] = 1 if k == g else 0
                bstack = fixed.tile([4, H], fp32)
                nc.sync.dma_start(bstack[:], d_bias.rearrange("h g -> g h"))
                onehot = fixed.tile([4, 4 * Bc], fp32)
                nc.vector.memset(onehot[:], 0.0)
                for g in range(4):
                    nc.vector.memset(onehot[g : g + 1, g * Bc : (g + 1) * Bc], 1.0)

            outbuf = outp.tile([B_CORE, T * O], fp32)

            hT = state.tile([H, B_CORE], fp32, tag="h")
            nc.sync.dma_start(hT[:], d_h0t[:])
            cT = state.tile([H, B_CORE], fp32, tag="c")
            nc.vector.memset(cT[:], 0.0)

            for t in range(T):
                gps = psum_pool.tile([128, 4 * Bc], fp32, tag="gpsum")
                if variant == 2:
                    # bias init: gps[m, g*Bc+b] = b_g[m]
                    nc.tensor.matmul(
                        gps[:], bstack[:], onehot[:], start=True, stop=False,
                        skip_group_check=True,
                    )
                    for g in range(4):
                        nc.tensor.matmul(
                            gps[:, g * Bc : (g + 1) * Bc],
                            wt[:, g * H : (g + 1) * H],
                            hT[:],
                            start=False,
                            stop=(g == 3),
                            skip_group_check=True,
                        )
                    # gate order i,f,o,g: one sigmoid over 3 gates
                    ifo = acts.tile([H, 3 * Bc], fp32, tag="ifo")
                    nc.scalar.activation(ifo[:], gps[:, 0 : 3 * Bc], AF.Sigmoid)
                    i_s = ifo[:, 0:Bc]
                    f_s = ifo[:, Bc : 2 * Bc]
                    o_s = ifo[:, 2 * Bc : 3 * Bc]
                    g_t = acts.tile([H, Bc], fp32, tag="g_t")
                    nc.scalar.activation(g_t[:], gps[:, 3 * Bc : 4 * Bc], AF.Tanh)
                else:
                    # gates^T: out[gate_row, b] — stationary = weight block
                    for g in range(4):
                        nc.tensor.matmul(
                            gps[:, g * Bc : (g + 1) * Bc],
                            wt[:, g * H : (g + 1) * H],
                            hT[:],
                            start=True,
                            stop=True,
                        )
                    i_sb = acts.tile([H, Bc], fp32, tag="i_s")
                    nc.scalar.activation(i_sb[:], gps[:, 0:Bc], AF.Sigmoid, bias=bias[:, 0:1])
                    f_sb = acts.tile([H, Bc], fp32, tag="f_s")
                    nc.scalar.activation(f_sb[:], gps[:, Bc : 2 * Bc], AF.Sigmoid, bias=bias[:, 1:2])
                    g_tb = acts.tile([H, Bc], fp32, tag="g_t")
                    nc.scalar.activation(g_tb[:], gps[:, 2 * Bc : 3 * Bc], AF.Tanh, bias=bias[:, 2:3])
                    o_sb = acts.tile([H, Bc], fp32, tag="o_s")
                    nc.scalar.activation(o_sb[:], gps[:, 3 * Bc : 4 * Bc], AF.Sigmoid, bias=bias[:, 3:4])
                    i_s, f_s, g_t, o_s = i_sb[:], f_sb[:], g_tb[:], o_sb[:]

                t1 = acts.tile([H, Bc], fp32, tag="t1")
                nc.vector.tensor_mul(t1[:], f_s, cT[:])
                t2 = acts.tile([H, Bc], fp32, tag="t2")
                nc.vector.tensor_mul(t2[:], i_s, g_t[:])
                cT = state.tile([H, Bc], fp32, tag="c")
                nc.vector.tensor_add(cT[:], t1[:], t2[:])

                th = acts.tile([H, Bc], fp32, tag="th")
                nc.scalar.activation(th[:], cT[:], AF.Tanh)
                hT = state.tile([H, Bc], fp32, tag="h")
                nc.vector.tensor_mul(hT[:], o_s, th[:])

                pps = ppsum_pool.tile([Bc, O], fp32, tag="ppsum")
                nc.tensor.matmul(pps[:], hT[:], woutt[:], start=True, stop=True)
                nc.vector.tensor_copy(outbuf[:, t * O : (t + 1) * O], pps[:])

            nc.sync.dma_start(d_preds[:], outbuf[:])

    return nc


def _get_program(T: int):
    key = (T, VARIANT)
    if key not in _PROGRAM_CACHE:
        _PROGRAM_CACHE[key] = _build_program(T)
    return _PROGRAM_CACHE[key]


def kernel(
    context_seq,
    W_ih,
    W_hh,
    b_ih,
    b_hh,
    W_out,
    b_out,
    prediction_len,
):
    from concourse.bass_utils import run_bass_kernel_spmd

    T = int(prediction_len)
    context_seq = np.asarray(context_seq, dtype=np.float32)
    W_ih = np.asarray(W_ih, dtype=np.float32)
    W_hh = np.asarray(W_hh, dtype=np.float32)
    b_ih = np.asarray(b_ih, dtype=np.float32)
    b_hh = np.asarray(b_hh, dtype=np.float32)
    W_out = np.asarray(W_out, dtype=np.float32)
    b_out = np.asarray(b_out, dtype=np.float32)

    B = context_seq.shape[0]
    assert B == B_TOTAL and context_seq.shape[2] == H

    # Host-side prep: only the last timestep of context_seq is used.
    h0 = context_seq[:, -1, :]  # [B, H]
    W = W_ih + W_hh  # [4H, H]
    b = b_ih + b_hh  # [4H]
    Wb = W.reshape(4, H, H)
    bb = b.reshape(4, H)
    if VARIANT == 2:
        order = [0, 1, 3, 2]  # i, f, o, g
        Wb = Wb[order]
        bb = bb[order]
    wt = np.ascontiguousarray(Wb.reshape(4 * H, H).T)  # [H, 4H]; col g*H+m = W[g block row m]
    bias_cols = np.ascontiguousarray(bb.T)  # [H, 4]
    woutt = np.ascontiguousarray(W_out.T)  # [H, O]

    nc = _get_program(T)

    in_maps = []
    for c in range(N_CORES):
        sh = h0[c * B_CORE : (c + 1) * B_CORE]  # [B_CORE, H]
        in_maps.append(
            {
                "h0t": np.ascontiguousarray(sh.T),  # [H, B_CORE]
                "wt": wt,
                "bias": bias_cols,
                "woutt": woutt,
            }
        )

    res = run_bass_kernel_spmd(nc, in_maps, core_ids=list(range(N_CORES)))

    out = np.empty((B_TOTAL, T, O), dtype=np.float32)
    for c in range(N_CORES):
        out[c * B_CORE : (c + 1) * B_CORE] = res.results[c]["preds"].reshape(
            B_CORE, T, O
        )
    out += b_out  # broadcast over [B, T, O]
    return out


# revision 18
# speedup vs baseline: 3.9923x; 3.9923x over previous
"""LSTM decoder kernel for Trainium2 (8 NeuronCores, data-parallel over batch).

Reference computation (per batch element b):
    h0 = context_seq[b, -1, :]          # only the LAST timestep is used
    c0 = 0
    for t in range(T):
        gates = h @ (W_ih + W_hh).T + (b_ih + b_hh)     # [4H], order i,f,g,o
        i, f, g, o = split(gates)
        c = sigmoid(f) * c + sigmoid(i) * tanh(g)
        h = sigmoid(o) * tanh(c)
        pred[t] = h @ W_out.T + b_out                   # [O]

Device layout (per core, B=128 batch rows): state kept TRANSPOSED — hT, cT
are [H=128 partitions, B free], so no per-step transposes are needed and
per-partition ACT bias lines up with gate rows. Per-step prediction via a
small matmul (stationary = hT) giving pred [B, 7] naturally, accumulated in
SBUF, one DMA at the end. b_out is added on the host.

Variants:
  1: single stream, per-gate ACT with per-partition bias (5 ACT instrs/step)
  4: single stream, gate order [i,f,o,g], i/f/o biases pre-added into PSUM
     by one K=3 one-hot matmul (prefetchable), merged sigmoid over i,f,o
     (3 ACT instrs/step); g-gate bias rides its tanh ACT.
  5: like 4 but TWO phase-offset streams of B/2 so the serial chains of the
     two halves overlap across engines.
"""

import json

import numpy as np

B_TOTAL = 1024
H = 128
O = 7
N_CORES = 8
B_CORE = B_TOTAL // N_CORES  # 128

VARIANT = 1

_N_STREAMS = {1: 1, 4: 1, 5: 2, 6: 2}
_STYLE = {1: "acts", 4: "biasmm", 5: "biasmm", 6: "acts"}


def _split_multiwait(bir_bytes: bytes) -> bytes:
    """This walrus build encodes at most ONE sync-wait per instruction.
    Split any multi-wait instruction into single-wait NoOps on the same
    engine (the sequencer executes them in program order, so waiting on
    each semaphore in turn is equivalent to waiting on all of them)."""
    bir = json.loads(bir_bytes)
    n = 0
    for f in bir.get("functions", []):
        for blk in f.get("blocks", []):
            new = []
            for inst in blk.get("instructions", []):
                si = inst.get("sync_info")
                waits = (si or {}).get("on_wait") or []
                if len(waits) > 1:
                    for w in waits[:-1]:
                        n += 1
                        nop = {
                            "name": f"WSPLIT-{n}",
                            "engine": inst.get("engine"),
                            "ins": [],
                            "outs": [],
                            "opcode": "NoOp",
                            "sync_info": {"on_update": [], "on_wait": [w]},
                        }
                        if inst.get("debug") is not None:
                            nop["debug"] = inst["debug"]
                        new.append(nop)
                    si["on_wait"] = [waits[-1]]
                new.append(inst)
            blk["instructions"] = new
    return json.dumps(bir).encode()


_PATCHED = False


def _patch_bass():
    global _PATCHED
    if _PATCHED:
        return
    import concourse.bass as bass

    orig = bass.Bass.to_json_bytes

    def patched(self, *a, **k):
        return _split_multiwait(orig(self, *a, **k))

    bass.Bass.to_json_bytes = patched
    _PATCHED = True


_PROGRAM_CACHE = {}


class _Stream:
    """Per-stream tiles + emit logic for one LSTM step."""

    def __init__(self, nc, tc, pools, consts, s, Bs, style):
        from concourse import mybir

        fp32 = mybir.dt.float32
        self.nc = nc
        self.s = s
        self.Bs = Bs
        self.style = style
        self.consts = consts
        self.state, self.acts, self.psum, self.ppsum, self.outp = pools
        self.h = None  # set by caller
        self.c = None
        self.outbuf = self.outp.tile(
            [Bs, consts["T"] * O], fp32, tag=f"outbuf{s}", name=f"outbuf{s}"
        )
        self.pred_pps = None  # 2-step batched pred psum tile

    def step(self, t):
        nc = self.nc
        from concourse import mybir

        fp32 = mybir.dt.float32
        AF = mybir.ActivationFunctionType
        s, Bs = self.s, self.Bs
        C = self.consts
        wt, bias, woutt = C["wt"], C["bias"], C["woutt"]

        if self.style == "biasmm":
            bstack, onehot = C["bstack"], C["onehot"]
            gp = self.psum.tile([128, 4 * Bs], fp32, tag=f"g{s}", bufs=2)
            # bias init for i,f,o cols [0:3Bs) — independent of h, prefetchable
            nc.tensor.matmul(gp[:, 0 : 3 * Bs], bstack[:], onehot[:],
                             start=True, stop=False, skip_group_check=True)
            for g in range(4):  # gate order in wt: i,f,o,g
                nc.tensor.matmul(
                    gp[:, g * Bs : (g + 1) * Bs],
                    wt[:, g * H : (g + 1) * H],
                    self.h[:],
                    start=False,
                    stop=(g == 3),
                    skip_group_check=True,
                )
            ifo = self.acts.tile([H, 3 * Bs], fp32, tag=f"ifo{s}", name=f"ifo{s}")
            nc.scalar.activation(ifo[:], gp[:, 0 : 3 * Bs], AF.Sigmoid)
            g_t = self.acts.tile([H, Bs], fp32, tag=f"gt{s}", name=f"gt{s}")
            nc.scalar.activation(g_t[:], gp[:, 3 * Bs : 4 * Bs], AF.Tanh,
                                 bias=bias[:, 3:4])
            i_s = ifo[:, 0:Bs]
            f_s = ifo[:, Bs : 2 * Bs]
            o_s = ifo[:, 2 * Bs : 3 * Bs]
        else:  # "acts": per-gate ACT with per-partition bias; wt order i,f,g,o
            gp = self.psum.tile([128, 4 * Bs], fp32, tag=f"g{s}", bufs=2)
            for g in (1, 0, 2, 3):  # emit f first: t1 depends on f alone
                nc.tensor.matmul(
                    gp[:, g * Bs : (g + 1) * Bs],
                    wt[:, g * H : (g + 1) * H],
                    self.h[:],
                    start=True,
                    stop=True,
                )
            f_t = self.acts.tile([H, Bs], fp32, tag=f"fs{s}", name=f"fs{s}")
            nc.scalar.activation(f_t[:], gp[:, Bs : 2 * Bs], AF.Sigmoid, bias=bias[:, 1:2])
            i_t = self.acts.tile([H, Bs], fp32, tag=f"is{s}", name=f"is{s}")
            nc.scalar.activation(i_t[:], gp[:, 0:Bs], AF.Sigmoid, bias=bias[:, 0:1])
            g_t = self.acts.tile([H, Bs], fp32, tag=f"gt{s}", name=f"gt{s}")
            nc.scalar.activation(g_t[:], gp[:, 2 * Bs : 3 * Bs], AF.Tanh, bias=bias[:, 2:3])
            o_t = self.acts.tile([H, Bs], fp32, tag=f"os{s}", name=f"os{s}")
            nc.scalar.activation(o_t[:], gp[:, 3 * Bs : 4 * Bs], AF.Sigmoid, bias=bias[:, 3:4])
            i_s, f_s, g_t, o_s = i_t[:], f_t[:], g_t, o_t[:]

        t1 = self.acts.tile([H, Bs], fp32, tag=f"t1{s}", name=f"t1{s}")
        nc.vector.tensor_mul(t1[:], f_s, self.c[:])
        t2 = self.acts.tile([H, Bs], fp32, tag=f"t2{s}", name=f"t2{s}")
        nc.vector.tensor_mul(t2[:], i_s, g_t[:])
        c_new = self.state.tile([H, Bs], fp32, tag=f"c{s}", name=f"c{s}")
        nc.vector.tensor_add(c_new[:], t1[:], t2[:])
        th = self.acts.tile([H, Bs], fp32, tag=f"th{s}", name=f"th{s}")
        nc.scalar.activation(th[:], c_new[:], AF.Tanh)
        h_new = self.state.tile([H, Bs], fp32, tag=f"h{s}", name=f"h{s}")
        nc.vector.tensor_mul(h_new[:], o_s, th[:])
        self.h, self.c = h_new, c_new

        # prediction: out [Bs, O] = h_new.T @ woutt; batch 2 steps per PSUM
        # tile + one DVE copy (an accumulation group writing disjoint halves)
        if t % 2 == 0:
            self.pred_pps = self.ppsum.tile([Bs, 2 * O], fp32, tag=f"pp{s}", bufs=2)
            nc.tensor.matmul(self.pred_pps[:, 0:O], h_new[:], woutt[:],
                             start=True, stop=False, skip_group_check=True)
            if t == self.consts["T"] - 1:  # odd T tail
                nc.vector.tensor_copy(self.outbuf[:, t * O : (t + 1) * O],
                                      self.pred_pps[:, 0:O])
        else:
            nc.tensor.matmul(self.pred_pps[:, O : 2 * O], h_new[:], woutt[:],
                             start=False, stop=True, skip_group_check=True)
            nc.vector.tensor_copy(self.outbuf[:, (t - 1) * O : (t + 1) * O],
                                  self.pred_pps[:])


def _build_program(T: int, variant: int = None, repeat: int = 1):
    if variant is None:
        variant = VARIANT
    import concourse.bass as bass
    import concourse.tile as tile
    from concourse import mybir

    _patch_bass()

    fp32 = mybir.dt.float32
    n_streams = _N_STREAMS[variant]
    style = _STYLE[variant]
    Bs = B_CORE // n_streams

    nc = bass.Bass("TRN2", debug=False)
    d_h0t = nc.dram_tensor("h0t", [H, B_CORE], fp32, kind="ExternalInput").ap()
    d_wt = nc.dram_tensor("wt", [H, 4 * H], fp32, kind="ExternalInput").ap()
    d_bias = nc.dram_tensor("bias", [H, 4], fp32, kind="ExternalInput").ap()
    d_woutt = nc.dram_tensor("woutt", [H, O], fp32, kind="ExternalInput").ap()
    if style == "biasmm":
        d_onehot = nc.dram_tensor("onehot", [3, 3 * Bs], fp32, kind="ExternalInput").ap()
    d_preds = nc.dram_tensor("preds", [B_CORE, T * O], fp32, kind="ExternalOutput").ap()

    with tile.TileContext(nc) as tc:
        with (
            tc.tile_pool(name="fixed", bufs=1) as fixed,
            tc.tile_pool(name="state", bufs=2) as state,
            tc.tile_pool(name="acts", bufs=2) as acts,
            tc.tile_pool(name="psum", bufs=2, space="PSUM") as psum_pool,
            tc.tile_pool(name="ppsum", bufs=2, space="PSUM") as ppsum_pool,
            tc.tile_pool(name="outp", bufs=1) as outp,
        ):
            consts = {"T": T}
            wt = fixed.tile([H, 4 * H], fp32)
            nc.sync.dma_start(wt[:], d_wt[:])
            bias = fixed.tile([H, 4], fp32)
            nc.sync.dma_start(bias[:], d_bias[:])
            woutt = fixed.tile([H, O], fp32)
            nc.sync.dma_start(woutt[:], d_woutt[:])
            consts.update(wt=wt, bias=bias, woutt=woutt)
            if style == "biasmm":
                bstack = fixed.tile([3, H], fp32)
                nc.sync.dma_start(bstack[:], d_bias.rearrange("h g -> g h")[0:3, :])
                onehot = fixed.tile([3, 3 * Bs], fp32)
                nc.sync.dma_start(onehot[:], d_onehot[:])
                consts.update(bstack=bstack, onehot=onehot)

            pools = (state, acts, psum_pool, ppsum_pool, outp)
            streams = [
                _Stream(nc, tc, pools, consts, s, Bs, style) for s in range(n_streams)
            ]
            # initial state
            h0s = []
            c0s = []
            for s, st in enumerate(streams):
                h0 = state.tile([H, Bs], fp32, tag=f"h{s}", name=f"h0_{s}")
                nc.sync.dma_start(h0[:], d_h0t[:, s * Bs : (s + 1) * Bs])
                c0 = state.tile([H, Bs], fp32, tag=f"c{s}", name=f"c0_{s}")
                nc.vector.memset(c0[:], 0.0)
                st.h, st.c = h0, c0
                h0s.append(h0)
                c0s.append(c0)

            def body():
                for t in range(T):
                    for st in streams:
                        st.step(t)

            if repeat > 1:
                with tc.For_i(0, repeat, 1):
                    body()
                    for s, st in enumerate(streams):
                        nc.vector.tensor_copy(h0s[s][:], st.h[:])
                        nc.vector.tensor_copy(c0s[s][:], st.c[:])
                        st.h, st.c = h0s[s], c0s[s]
            else:
                body()

            for s, st in enumerate(streams):
                nc.sync.dma_start(d_preds[s * Bs : (s + 1) * Bs, :], st.outbuf[:])

    return nc


_RUNNER_CACHE = {}


def _get_runner(nc):
    """Build (once per program) a jitted shard_map callable over the 8 cores.
    run_bass_kernel_spmd rebuilds its jit closure every call, which retraces
    and re-lowers (including BIR serialization) each time — ~1-2.5s of
    client-side overhead per invocation. Caching the jitted callable makes
    repeat invocations cheap."""
    key = id(nc)
    if key in _RUNNER_CACHE:
        return _RUNNER_CACHE[key]

    import jax
    import numpy as np_
    from jax.sharding import Mesh, PartitionSpec
    from jax.experimental.shard_map import shard_map
    import concourse.mybir as mybir
    from concourse.bass2jax import (
        _bass_exec_p,
        install_neuronx_cc_hook,
        partition_id_tensor,
    )

    install_neuronx_cc_hook()

    partition_name = nc.partition_id_tensor.name if nc.partition_id_tensor else None
    in_names = []
    out_names = []
    out_avals = []
    zero_shapes = []
    for alloc in nc.m.functions[0].allocations:
        if not isinstance(alloc, mybir.MemoryLocationSet):
            continue
        name = alloc.memorylocations[0].name
        if alloc.kind == "ExternalInput":
            if name != partition_name:
                in_names.append(name)
        elif alloc.kind == "ExternalOutput":
            shape = tuple(alloc.tensor_shape)
            dtype = mybir.dt.np(alloc.dtype)
            out_names.append(name)
            out_avals.append(jax.core.ShapedArray(shape, dtype))
            zero_shapes.append((shape, dtype))
    n_params = len(in_names)
    n_outs = len(out_names)
    all_in_names = list(in_names) + list(out_names)
    if partition_name is not None:
        all_in_names.append(partition_name)

    def _body(*args):
        operands = list(args)
        if partition_name is not None:
            operands.append(partition_id_tensor())
        outs = _bass_exec_p.bind(
            *operands,
            out_avals=tuple(out_avals),
            in_names=tuple(all_in_names),
            out_names=tuple(out_names),
            lowering_input_output_aliases=(),
            sim_require_finite=True,
            sim_require_nnan=True,
            nc=nc,
        )
        return tuple(outs)

    donate = tuple(range(n_params, n_params + n_outs))
    devices = jax.devices()[:N_CORES]
    mesh = Mesh(np_.asarray(devices), ("core",))
    in_specs = (PartitionSpec("core"),) * (n_params + n_outs)
    out_specs = (PartitionSpec("core"),) * n_outs
    sharded = jax.jit(
        shard_map(_body, mesh=mesh, in_specs=in_specs, out_specs=out_specs, check_rep=False),
        donate_argnums=donate,
        keep_unused=True,
    )

    def run(in_maps):
        per_core = [[np.asarray(m[name]) for name in in_names] for m in in_maps]
        concat_in = [
            np.concatenate([per_core[c][i] for c in range(N_CORES)], axis=0)
            for i in range(n_params)
        ]
        concat_zeros = [np.zeros((N_CORES * s[0], *s[1:]), d) for s, d in zero_shapes]
        out_arrs = sharded(*concat_in, *concat_zeros)
        return [
            {
                name: np.asarray(out_arrs[i]).reshape(N_CORES, *out_avals[i].shape)[c]
                for i, name in enumerate(out_names)
            }
            for c in range(N_CORES)
        ]

    _RUNNER_CACHE[key] = run
    return run


def _onehot_input(variant):
    if _STYLE[variant] != "biasmm":
        return None
    Bs = B_CORE // _N_STREAMS[variant]
    oh = np.zeros((3, 3 * Bs), dtype=np.float32)
    for g in range(3):
        oh[g, g * Bs : (g + 1) * Bs] = 1.0
    return oh


def _gate_order(variant):
    # order of gate blocks in the wt layout
    return [0, 1, 3, 2] if _STYLE[variant] == "biasmm" else [0, 1, 2, 3]


def _get_program(T: int):
    key = (T, VARIANT)
    if key not in _PROGRAM_CACHE:
        _PROGRAM_CACHE[key] = _build_program(T)
    return _PROGRAM_CACHE[key]


def kernel(
    context_seq,
    W_ih,
    W_hh,
    b_ih,
    b_hh,
    W_out,
    b_out,
    prediction_len,
):
    T = int(prediction_len)
    context_seq = np.asarray(context_seq, dtype=np.float32)
    W_ih = np.asarray(W_ih, dtype=np.float32)
    W_hh = np.asarray(W_hh, dtype=np.float32)
    b_ih = np.asarray(b_ih, dtype=np.float32)
    b_hh = np.asarray(b_hh, dtype=np.float32)
    W_out = np.asarray(W_out, dtype=np.float32)
    b_out = np.asarray(b_out, dtype=np.float32)

    B = context_seq.shape[0]
    assert B == B_TOTAL and context_seq.shape[2] == H

    # Host-side prep: only the last timestep of context_seq is used.
    h0 = context_seq[:, -1, :]  # [B, H]
    W = W_ih + W_hh  # [4H, H]
    b = b_ih + b_hh  # [4H]
    order = _gate_order(VARIANT)
    Wb = W.reshape(4, H, H)[order]
    bb = b.reshape(4, H)[order]
    wt = np.ascontiguousarray(Wb.reshape(4 * H, H).T)  # [H, 4H]
    bias_cols = np.ascontiguousarray(bb.T)  # [H, 4]
    woutt = np.ascontiguousarray(W_out.T)  # [H, O]

    nc = _get_program(T)

    in_maps = []
    for c in range(N_CORES):
        sh = h0[c * B_CORE : (c + 1) * B_CORE]  # [B_CORE, H]
        m = {
            "h0t": np.ascontiguousarray(sh.T),  # [H, B_CORE]
            "wt": wt,
            "bias": bias_cols,
            "woutt": woutt,
        }
        oh = _onehot_input(VARIANT)
        if oh is not None:
            m["onehot"] = oh
        in_maps.append(m)

    results = _get_runner(nc)(in_maps)

    out = np.empty((B_TOTAL, T, O), dtype=np.float32)
    for c in range(N_CORES):
        out[c * B_CORE : (c + 1) * B_CORE] = results[c]["preds"].reshape(B_CORE, T, O)
    out += b_out  # broadcast over [B, T, O]
    return out


# revision 26
# speedup vs baseline: 4.0954x; 1.0258x over previous
"""LSTM decoder kernel for Trainium2 (8 NeuronCores, data-parallel over batch).

Reference computation (per batch element b):
    h0 = context_seq[b, -1, :]          # only the LAST timestep is used
    c0 = 0
    for t in range(T):
        gates = h @ (W_ih + W_hh).T + (b_ih + b_hh)     # [4H], order i,f,g,o
        i, f, g, o = split(gates)
        c = sigmoid(f) * c + sigmoid(i) * tanh(g)
        h = sigmoid(o) * tanh(c)
        pred[t] = h @ W_out.T + b_out                   # [O]

Device layout (per core, B=128 batch rows): state kept TRANSPOSED — hT, cT
are [H=128 partitions, B free], so no per-step transposes are needed and
per-partition ACT bias lines up with gate rows. Per-step prediction via a
small matmul (stationary = hT) giving pred [B, 7] naturally, accumulated in
SBUF, one DMA at the end. b_out is added on the host.

Variants (HW-measured per-step time at T=512, 8 cores):
  1: single stream, per-gate ACT bias, one gates PSUM bank   (4.2 us/step)
  4: merged sigmoid via K=3 one-hot bias matmul              (5.5 us/step)
  5: variant 4 x two phase-offset streams of B/2             (slow)
  6: variant 1 x two phase-offset streams of B/2             (5.8 us/step)
  7: per-gate ACT bias, gates split into {f,i}/{g,o} PSUM banks so
     sigmoid(f) starts after two matmuls; predictions batched 4 steps per
     PSUM tile/copy                                          (3.4 us/step) <- default
  8: one PSUM bank per gate                                  (4.1 us/step)
"""

import json

import numpy as np

B_TOTAL = 1024
H = 128
O = 7
N_CORES = 8
B_CORE = B_TOTAL // N_CORES  # 128

VARIANT = 7

ACTS_BUFS = 2
_N_STREAMS = {1: 1, 4: 1, 5: 2, 6: 2, 7: 1, 8: 1}
_STYLE = {1: "acts", 4: "biasmm", 5: "biasmm", 6: "acts", 7: "acts2", 8: "acts4"}


def _split_multiwait(bir_bytes: bytes) -> bytes:
    """This walrus build encodes at most ONE sync-wait per instruction.
    Split any multi-wait instruction into single-wait NoOps on the same
    engine (the sequencer executes them in program order, so waiting on
    each semaphore in turn is equivalent to waiting on all of them)."""
    bir = json.loads(bir_bytes)
    n = 0
    for f in bir.get("functions", []):
        for blk in f.get("blocks", []):
            new = []
            for inst in blk.get("instructions", []):
                si = inst.get("sync_info")
                waits = (si or {}).get("on_wait") or []
                if len(waits) > 1:
                    for w in waits[:-1]:
                        n += 1
                        nop = {
                            "name": f"WSPLIT-{n}",
                            "engine": inst.get("engine"),
                            "ins": [],
                            "outs": [],
                            "opcode": "NoOp",
                            "sync_info": {"on_update": [], "on_wait": [w]},
                        }
                        if inst.get("debug") is not None:
                            nop["debug"] = inst["debug"]
                        new.append(nop)
                    si["on_wait"] = [waits[-1]]
                new.append(inst)
            blk["instructions"] = new
    return json.dumps(bir).encode()


_PATCHED = False


def _patch_bass():
    global _PATCHED
    if _PATCHED:
        return
    import concourse.bass as bass

    orig = bass.Bass.to_json_bytes

    def patched(self, *a, **k):
        return _split_multiwait(orig(self, *a, **k))

    bass.Bass.to_json_bytes = patched
    _PATCHED = True


_PROGRAM_CACHE = {}


class _Stream:
    """Per-stream tiles + emit logic for one LSTM step."""

    def __init__(self, nc, tc, pools, consts, s, Bs, style):
        from concourse import mybir

        fp32 = mybir.dt.float32
        self.nc = nc
        self.s = s
        self.Bs = Bs
        self.style = style
        self.consts = consts
        self.state, self.acts, self.psum, self.ppsum, self.outp = pools
        self.h = None  # set by caller
        self.c = None
        self.outbuf = self.outp.tile(
            [Bs, consts["T"] * O], fp32, tag=f"outbuf{s}", name=f"outbuf{s}"
        )
        self.pred_pps = None  # 2-step batched pred psum tile

    def step(self, t):
        nc = self.nc
        from concourse import mybir

        fp32 = mybir.dt.float32
        AF = mybir.ActivationFunctionType
        s, Bs = self.s, self.Bs
        C = self.consts
        wt, bias, woutt = C["wt"], C["bias"], C["woutt"]

        if self.style == "biasmm":
            bstack, onehot = C["bstack"], C["onehot"]
            gp = self.psum.tile([128, 4 * Bs], fp32, tag=f"g{s}", bufs=2)
            # bias init for i,f,o cols [0:3Bs) — independent of h, prefetchable
            nc.tensor.matmul(gp[:, 0 : 3 * Bs], bstack[:], onehot[:],
                             start=True, stop=False, skip_group_check=True)
            for g in range(4):  # gate order in wt: i,f,o,g
                nc.tensor.matmul(
                    gp[:, g * Bs : (g + 1) * Bs],
                    wt[:, g * H : (g + 1) * H],
                    self.h[:],
                    start=False,
                    stop=(g == 3),
                    skip_group_check=True,
                )
            ifo = self.acts.tile([H, 3 * Bs], fp32, tag=f"ifo{s}", name=f"ifo{s}")
            nc.scalar.activation(ifo[:], gp[:, 0 : 3 * Bs], AF.Sigmoid)
            g_t = self.acts.tile([H, Bs], fp32, tag=f"gt{s}", name=f"gt{s}")
            nc.scalar.activation(g_t[:], gp[:, 3 * Bs : 4 * Bs], AF.Tanh,
                                 bias=bias[:, 3:4])
            i_s = ifo[:, 0:Bs]
            f_s = ifo[:, Bs : 2 * Bs]
            o_s = ifo[:, 2 * Bs : 3 * Bs]
        elif self.style == "acts4":
            # one PSUM bank per gate: each sigmoid starts right after its own
            # matmul; wt order i,f,g,o; ACT order f,i,g,o
            banks = {}
            for g, tag, bufs in ((1, "bf", 2), (0, "bi", 2), (2, "bg", 1), (3, "bo", 1)):
                pb = self.psum.tile([128, Bs], fp32, tag=f"{tag}{s}", bufs=bufs)
                nc.tensor.matmul(pb[:], wt[:, g * H : (g + 1) * H], self.h[:],
                                 start=True, stop=True)
                banks[g] = pb
            f_t = self.acts.tile([H, Bs], fp32, tag=f"fs{s}", name=f"fs{s}")
            nc.scalar.activation(f_t[:], banks[1][:], AF.Sigmoid, bias=bias[:, 1:2])
            i_t = self.acts.tile([H, Bs], fp32, tag=f"is{s}", name=f"is{s}")
            nc.scalar.activation(i_t[:], banks[0][:], AF.Sigmoid, bias=bias[:, 0:1])
            g_t = self.acts.tile([H, Bs], fp32, tag=f"gt{s}", name=f"gt{s}")
            nc.scalar.activation(g_t[:], banks[2][:], AF.Tanh, bias=bias[:, 2:3])
            o_t = self.acts.tile([H, Bs], fp32, tag=f"os{s}", name=f"os{s}")
            nc.scalar.activation(o_t[:], banks[3][:], AF.Sigmoid, bias=bias[:, 3:4])
            i_s, f_s, g_t, o_s = i_t[:], f_t[:], g_t, o_t[:]
        elif self.style == "acts2":
            # per-gate ACT bias, but gates split across TWO PSUM banks
            # ({f,i} and {g,o}) so sigmoid(f) starts after two matmuls
            # instead of four; wt order i,f,g,o
            gfi = self.psum.tile([128, 2 * Bs], fp32, tag=f"gfi{s}", bufs=2)
            ggo = self.psum.tile([128, 2 * Bs], fp32, tag=f"ggo{s}", bufs=1)
            for g, dst, col in ((1, gfi, 0), (0, gfi, 1), (2, ggo, 0), (3, ggo, 1)):
                nc.tensor.matmul(
                    dst[:, col * Bs : (col + 1) * Bs],
                    wt[:, g * H : (g + 1) * H],
                    self.h[:],
                    start=True,
                    stop=True,
                )
            f_t = self.acts.tile([H, Bs], fp32, tag=f"fs{s}", name=f"fs{s}")
            nc.scalar.activation(f_t[:], gfi[:, 0:Bs], AF.Sigmoid, bias=bias[:, 1:2])
            i_t = self.acts.tile([H, Bs], fp32, tag=f"is{s}", name=f"is{s}")
            nc.scalar.activation(i_t[:], gfi[:, Bs : 2 * Bs], AF.Sigmoid, bias=bias[:, 0:1])
            g_t = self.acts.tile([H, Bs], fp32, tag=f"gt{s}", name=f"gt{s}")
            nc.scalar.activation(g_t[:], ggo[:, 0:Bs], AF.Tanh, bias=bias[:, 2:3])
            o_t = self.acts.tile([H, Bs], fp32, tag=f"os{s}", name=f"os{s}")
            nc.scalar.activation(o_t[:], ggo[:, Bs : 2 * Bs], AF.Sigmoid, bias=bias[:, 3:4])
            i_s, f_s, g_t, o_s = i_t[:], f_t[:], g_t, o_t[:]
        else:  # "acts": per-gate ACT with per-partition bias; wt order i,f,g,o
            gp = self.psum.tile([128, 4 * Bs], fp32, tag=f"g{s}", bufs=2)
            for g in (1, 0, 2, 3):  # emit f first: t1 depends on f alone
                nc.tensor.matmul(
                    gp[:, g * Bs : (g + 1) * Bs],
                    wt[:, g * H : (g + 1) * H],
                    self.h[:],
                    start=True,
                    stop=True,
                )
            f_t = self.acts.tile([H, Bs], fp32, tag=f"fs{s}", name=f"fs{s}")
            nc.scalar.activation(f_t[:], gp[:, Bs : 2 * Bs], AF.Sigmoid, bias=bias[:, 1:2])
            i_t = self.acts.tile([H, Bs], fp32, tag=f"is{s}", name=f"is{s}")
            nc.scalar.activation(i_t[:], gp[:, 0:Bs], AF.Sigmoid, bias=bias[:, 0:1])
            g_t = self.acts.tile([H, Bs], fp32, tag=f"gt{s}", name=f"gt{s}")
            nc.scalar.activation(g_t[:], gp[:, 2 * Bs : 3 * Bs], AF.Tanh, bias=bias[:, 2:3])
            o_t = self.acts.tile([H, Bs], fp32, tag=f"os{s}", name=f"os{s}")
            nc.scalar.activation(o_t[:], gp[:, 3 * Bs : 4 * Bs], AF.Sigmoid, bias=bias[:, 3:4])
            i_s, f_s, g_t, o_s = i_t[:], f_t[:], g_t, o_t[:]

        t1 = self.acts.tile([H, Bs], fp32, tag=f"t1{s}", name=f"t1{s}")
        nc.vector.tensor_mul(t1[:], f_s, self.c[:])
        t2 = self.acts.tile([H, Bs], fp32, tag=f"t2{s}", name=f"t2{s}")
        nc.vector.tensor_mul(t2[:], i_s, g_t[:])
        c_new = self.state.tile([H, Bs], fp32, tag=f"c{s}", name=f"c{s}")
        nc.vector.tensor_add(c_new[:], t1[:], t2[:])
        th = self.acts.tile([H, Bs], fp32, tag=f"th{s}", name=f"th{s}")
        nc.scalar.activation(th[:], c_new[:], AF.Tanh)
        h_new = self.state.tile([H, Bs], fp32, tag=f"h{s}", name=f"h{s}")
        nc.vector.tensor_mul(h_new[:], o_s, th[:])
        self.h, self.c = h_new, c_new

        # prediction: out [Bs, O] = h_new.T @ woutt; batch PB steps per PSUM
        # tile + one DVE copy (an accumulation group writing disjoint slots)
        PB = 4 if self.style in ("acts2", "acts4") else 2
        k = t % PB
        if k == 0:
            self.pred_pps = self.ppsum.tile([Bs, PB * O], fp32, tag=f"pp{s}", bufs=2)
        nc.tensor.matmul(self.pred_pps[:, k * O : (k + 1) * O], h_new[:], woutt[:],
                         start=(k == 0), stop=(k == PB - 1), skip_group_check=True)
        if k == PB - 1 or t == self.consts["T"] - 1:
            nc.vector.tensor_copy(
                self.outbuf[:, (t - k) * O : (t + 1) * O],
                self.pred_pps[:, 0 : (k + 1) * O],
            )


def _build_program(T: int, variant: int = None, repeat: int = 1):
    if variant is None:
        variant = VARIANT
    import concourse.bass as bass
    import concourse.tile as tile
    from concourse import mybir

    _patch_bass()

    fp32 = mybir.dt.float32
    n_streams = _N_STREAMS[variant]
    style = _STYLE[variant]
    Bs = B_CORE // n_streams

    nc = bass.Bass("TRN2", debug=False)
    d_h0t = nc.dram_tensor("h0t", [H, B_CORE], fp32, kind="ExternalInput").ap()
    d_wt = nc.dram_tensor("wt", [H, 4 * H], fp32, kind="ExternalInput").ap()
    d_bias = nc.dram_tensor("bias", [H, 4], fp32, kind="ExternalInput").ap()
    d_woutt = nc.dram_tensor("woutt", [H, O], fp32, kind="ExternalInput").ap()
    if style == "biasmm":
        d_onehot = nc.dram_tensor("onehot", [3, 3 * Bs], fp32, kind="ExternalInput").ap()
    d_preds = nc.dram_tensor("preds", [B_CORE, T * O], fp32, kind="ExternalOutput").ap()

    with tile.TileContext(nc) as tc:
        with (
            tc.tile_pool(name="fixed", bufs=1) as fixed,
            tc.tile_pool(name="state", bufs=2) as state,
            tc.tile_pool(name="acts", bufs=ACTS_BUFS) as acts,
            tc.tile_pool(name="psum", bufs=2, space="PSUM") as psum_pool,
            tc.tile_pool(name="ppsum", bufs=2, space="PSUM") as ppsum_pool,
            tc.tile_pool(name="outp", bufs=1) as outp,
        ):
            consts = {"T": T}
            wt = fixed.tile([H, 4 * H], fp32)
            nc.sync.dma_start(wt[:], d_wt[:])
            bias = fixed.tile([H, 4], fp32)
            nc.sync.dma_start(bias[:], d_bias[:])
            woutt = fixed.tile([H, O], fp32)
            nc.sync.dma_start(woutt[:], d_woutt[:])
            consts.update(wt=wt, bias=bias, woutt=woutt)
            if style == "biasmm":
                bstack = fixed.tile([3, H], fp32)
                nc.sync.dma_start(bstack[:], d_bias.rearrange("h g -> g h")[0:3, :])
                onehot = fixed.tile([3, 3 * Bs], fp32)
                nc.sync.dma_start(onehot[:], d_onehot[:])
                consts.update(bstack=bstack, onehot=onehot)

            pools = (state, acts, psum_pool, ppsum_pool, outp)
            streams = [
                _Stream(nc, tc, pools, consts, s, Bs, style) for s in range(n_streams)
            ]
            # initial state
            h0s = []
            c0s = []
            for s, st in enumerate(streams):
                h0 = state.tile([H, Bs], fp32, tag=f"h{s}", name=f"h0_{s}")
                nc.sync.dma_start(h0[:], d_h0t[:, s * Bs : (s + 1) * Bs])
                c0 = state.tile([H, Bs], fp32, tag=f"c{s}", name=f"c0_{s}")
                nc.vector.memset(c0[:], 0.0)
                st.h, st.c = h0, c0
                h0s.append(h0)
                c0s.append(c0)

            def body():
                for t in range(T):
                    for st in streams:
                        st.step(t)

            if repeat > 1:
                with tc.For_i(0, repeat, 1):
                    body()
                    for s, st in enumerate(streams):
                        nc.vector.tensor_copy(h0s[s][:], st.h[:])
                        nc.vector.tensor_copy(c0s[s][:], st.c[:])
                        st.h, st.c = h0s[s], c0s[s]
            else:
                body()

            for s, st in enumerate(streams):
                nc.sync.dma_start(d_preds[s * Bs : (s + 1) * Bs, :], st.outbuf[:])

    return nc


_RUNNER_CACHE = {}


def _get_runner(nc):
    """Build (once per program) a jitted shard_map callable over the 8 cores.
    run_bass_kernel_spmd rebuilds its jit closure every call, which retraces
    and re-lowers (including BIR serialization) each time — ~1-2.5s of
    client-side overhead per invocation. Caching the jitted callable makes
    repeat invocations cheap."""
    key = id(nc)
    if key in _RUNNER_CACHE:
        return _RUNNER_CACHE[key]

    import jax
    import numpy as np_
    from jax.sharding import Mesh, PartitionSpec
    from jax.experimental.shard_map import shard_map
    import concourse.mybir as mybir
    from concourse.bass2jax import (
        _bass_exec_p,
        install_neuronx_cc_hook,
        partition_id_tensor,
    )

    install_neuronx_cc_hook()

    partition_name = nc.partition_id_tensor.name if nc.partition_id_tensor else None
    in_names = []
    out_names = []
    out_avals = []
    zero_shapes = []
    for alloc in nc.m.functions[0].allocations:
        if not isinstance(alloc, mybir.MemoryLocationSet):
            continue
        name = alloc.memorylocations[0].name
        if alloc.kind == "ExternalInput":
            if name != partition_name:
                in_names.append(name)
        elif alloc.kind == "ExternalOutput":
            shape = tuple(alloc.tensor_shape)
            dtype = mybir.dt.np(alloc.dtype)
            out_names.append(name)
            out_avals.append(jax.core.ShapedArray(shape, dtype))
            zero_shapes.append((shape, dtype))
    n_params = len(in_names)
    n_outs = len(out_names)
    all_in_names = list(in_names) + list(out_names)
    if partition_name is not None:
        all_in_names.append(partition_name)

    def _body(*args):
        operands = list(args)
        if partition_name is not None:
            operands.append(partition_id_tensor())
        outs = _bass_exec_p.bind(
            *operands,
            out_avals=tuple(out_avals),
            in_names=tuple(all_in_names),
            out_names=tuple(out_names),
            lowering_input_output_aliases=(),
            sim_require_finite=True,
            sim_require_nnan=True,
            nc=nc,
        )
        return tuple(outs)

    donate = tuple(range(n_params, n_params + n_outs))
    devices = jax.devices()[:N_CORES]
    mesh = Mesh(np_.asarray(devices), ("core",))
    in_specs = (PartitionSpec("core"),) * (n_params + n_outs)
    out_specs = (PartitionSpec("core"),) * n_outs
    sharded = jax.jit(
        shard_map(_body, mesh=mesh, in_specs=in_specs, out_specs=out_specs, check_rep=False),
        donate_argnums=donate,
        keep_unused=True,
    )

    def run(in_maps):
        per_core = [[np.asarray(m[name]) for name in in_names] for m in in_maps]
        concat_in = [
            np.concatenate([per_core[c][i] for c in range(N_CORES)], axis=0)
            for i in range(n_params)
        ]
        concat_zeros = [np.zeros((N_CORES * s[0], *s[1:]), d) for s, d in zero_shapes]
        out_arrs = sharded(*concat_in, *concat_zeros)
        return [
            {
                name: np.asarray(out_arrs[i]).reshape(N_CORES, *out_avals[i].shape)[c]
                for i, name in enumerate(out_names)
            }
            for c in range(N_CORES)
        ]

    _RUNNER_CACHE[key] = run
    return run


def _onehot_input(variant):
    if _STYLE[variant] != "biasmm":
        return None
    Bs = B_CORE // _N_STREAMS[variant]
    oh = np.zeros((3, 3 * Bs), dtype=np.float32)
    for g in range(3):
        oh[g, g * Bs : (g + 1) * Bs] = 1.0
    return oh


def _gate_order(variant):
    # order of gate blocks in the wt layout
    return [0, 1, 3, 2] if _STYLE[variant] == "biasmm" else [0, 1, 2, 3]


def _get_program(T: int):
    key = (T, VARIANT)
    if key not in _PROGRAM_CACHE:
        _PROGRAM_CACHE[key] = _build_program(T)
    return _PROGRAM_CACHE[key]


def kernel(
    context_seq,
    W_ih,
    W_hh,
    b_ih,
    b_hh,
    W_out,
    b_out,
    prediction_len,
):
    T = int(prediction_len)
    context_seq = np.asarray(context_seq, dtype=np.float32)
    W_ih = np.asarray(W_ih, dtype=np.float32)
    W_hh = np.asarray(W_hh, dtype=np.float32)
    b_ih = np.asarray(b_ih, dtype=np.float32)
    b_hh = np.asarray(b_hh, dtype=np.float32)
    W_out = np.asarray(W_out, dtype=np.float32)
    b_out = np.asarray(b_out, dtype=np.float32)

    B = context_seq.shape[0]
    assert B == B_TOTAL and context_seq.shape[2] == H

    # Host-side prep: only the last timestep of context_seq is used.
    h0 = context_seq[:, -1, :]  # [B, H]
    W = W_ih + W_hh  # [4H, H]
    b = b_ih + b_hh  # [4H]
    order = _gate_order(VARIANT)
    Wb = W.reshape(4, H, H)[order]
    bb = b.reshape(4, H)[order]
    wt = np.ascontiguousarray(Wb.reshape(4 * H, H).T)  # [H, 4H]
    bias_cols = np.ascontiguousarray(bb.T)  # [H, 4]
    woutt = np.ascontiguousarray(W_out.T)  # [H, O]

    nc = _get_program(T)

    in_maps = []
    for c in range(N_CORES):
        sh = h0[c * B_CORE : (c + 1) * B_CORE]  # [B_CORE, H]
        m = {
            "h0t": np.ascontiguousarray(sh.T),  # [H, B_CORE]
            "wt": wt,
            "bias": bias_cols,
            "woutt": woutt,
        }
        oh = _onehot_input(VARIANT)
        if oh is not None:
            m["onehot"] = oh
        in_maps.append(m)

    results = _get_runner(nc)(in_maps)

    out = np.empty((B_TOTAL, T, O), dtype=np.float32)
    for c in range(N_CORES):
        out[c * B_CORE : (c + 1) * B_CORE] = results[c]["preds"].reshape(B_CORE, T, O)
    out += b_out  # broadcast over [B, T, O]
    return out
